# revision 1
# baseline (speedup 1.0000x reference)
"""EMD loss kernel for Trainium2 (8 NeuronCores, pure data parallel).

Computes out[b] = sum_t (cumsum(x-y, axis=1)[b, t])^2 for x, y [131072, 256] f32.

Per-core plan (16384 rows each, no cross-core communication):
  - x and y are packed host-side into one [2, 16384, 256] DRAM parameter so
    each streaming chunk is ONE DMA instruction on the qSP HWDGE ring (the
    ring executes its DMAs serially — fewer/bigger instructions = more ring
    throughput — and x/y arrive together).
  - View the shard as [128 partitions, 128 row-blocks, 256 bins]; per
    row-block a VectorE tensor_tensor_scan computes the running CDF
    difference state = (x_t + state) - y_t in ONE instruction, then a ScalarE
    activation(Square, accum_out=...) squares and row-sums in ONE instruction.
  - Tail chunks taper (8,4,2,1,1 blocks) and live in their own pool tags so
    the ring never stalls on slot releases at the end; trailing compute after
    the last DMA is ~1 row-block.
"""

import numpy as np

from concourse import bacc, bass, mybir
from concourse.bass_utils import run_bass_kernel_spmd
from concourse.tile import TileContext

N_CORES = 8
B = 131072
BINS = 256
ROWS = B // N_CORES  # 16384 rows per core
P = 128  # SBUF partitions
N_BLK = ROWS // P  # 128 row-blocks per core (one row per partition each)
# 8-row-block (2 MB) streaming chunks pipeline the DMA-completion semaphore
# latency best (measured vs 4/16/32-block variants); the tail tapers in
# dedicated pool slots so trailing compute after the last DMA is ~1 block.
HEAD = [8] * 14  # main-pool streaming chunks
CHUNK_SLOT = 8  # main io pool slot size in row-blocks
IO_BUFS = 8
TAIL = [8, 4, 2, 1, 1]  # dedicated slots each
CHUNKS = HEAD + TAIL
assert sum(CHUNKS) == N_BLK
C_BUFS = 16
SQ_BUFS = 8

F32 = mybir.dt.float32


def build_nc() -> bass.Bass:
    nc = bacc.Bacc()

    xy = nc.declare_dram_parameter("xy", [2, ROWS, BINS], F32, isOutput=False)
    out = nc.declare_dram_parameter("out", [ROWS], F32, isOutput=True)

    # [128, 2, N_BLK * BINS]; partition p holds rows p*N_BLK .. p*N_BLK+N_BLK-1
    xyv = xy[:].rearrange("z (p n) d -> p z (n d)", p=P)
    ov = out[:].rearrange("(p n) -> p n", p=P)  # [128, N_BLK]

    with (
        TileContext(nc) as tc,
        tc.tile_pool(name="io", bufs=IO_BUFS) as io_pool,
        tc.tile_pool(name="iotail", bufs=1) as tail_pool,
        tc.tile_pool(name="cdf", bufs=C_BUFS) as c_pool,
        tc.tile_pool(name="res", bufs=1) as res_pool,
        tc.tile_pool(name="sq", bufs=SQ_BUFS, space="PSUM") as sq_pool,
    ):
        out_sb = res_pool.tile([P, N_BLK], F32)

        # Warm the ACT Square table at t=0 so the ~2.7us table load overlaps
        # the first input DMAs instead of stalling the first real activation.
        warm = res_pool.tile([P, 1], F32, tag="warm")
        warm2 = res_pool.tile([P, 1], F32, tag="warm2")
        nc.vector.memset(warm[:], 0)
        nc.scalar.activation(
            out=warm2[:],
            in_=warm[:],
            func=mybir.ActivationFunctionType.Square,
        )

        blk0 = 0
        for ci, tsz in enumerate(CHUNKS):
            if ci < len(HEAD):
                slot = CHUNK_SLOT
                xyt = io_pool.tile(
                    [P, 2 * slot * BINS], F32, tag="xyt", name=f"xyt{ci}"
                )
            else:
                slot = tsz
                xyt = tail_pool.tile(
                    [P, 2 * slot * BINS], F32, tag=f"tail{ci}", name=f"xyt{ci}"
                )
            # [128, 2, tsz*256] view of the slot: x at free offset 0, y at
            # slot*BINS — matches the DRAM [p, z, f] chunk below.
            xyt3 = xyt[:].rearrange("p (z f) -> p z f", z=2)[:, :, : tsz * BINS]
            lo, hi = blk0 * BINS, (blk0 + tsz) * BINS
            nc.sync.dma_start(out=xyt3, in_=xyv[:, :, lo:hi])
            for t in range(tsz):
                col = blk0 + t
                xoff = t * BINS
                yoff = slot * BINS + t * BINS
                c = c_pool.tile([P, BINS], F32)
                nc.vector.tensor_tensor_scan(
                    out=c[:],
                    data0=xyt[:, xoff : xoff + BINS],
                    data1=xyt[:, yoff : yoff + BINS],
                    initial=0.0,
                    op0=mybir.AluOpType.add,
                    op1=mybir.AluOpType.subtract,
                )
                sq = sq_pool.tile([P, BINS], F32)
                nc.scalar.activation(
                    out=sq[:],
                    in_=c[:],
                    func=mybir.ActivationFunctionType.Square,
                    accum_out=out_sb[:, col : col + 1],
                )
            blk0 += tsz
        nc.sync.dma_start(out=ov[:, :], in_=out_sb[:])
    nc.finalize()
    return nc


_NC = None


def _get_nc() -> bass.Bass:
    global _NC
    if _NC is None:
        _NC = build_nc()
    return _NC


def kernel(x: np.ndarray, y: np.ndarray) -> np.ndarray:
    assert x.shape == (B, BINS) and y.shape == (B, BINS), (x.shape, y.shape)
    x = np.ascontiguousarray(x, dtype=np.float32)
    y = np.ascontiguousarray(y, dtype=np.float32)
    in_maps = []
    for i in range(N_CORES):
        sl = slice(i * ROWS, (i + 1) * ROWS)
        in_maps.append({"xy": np.stack([x[sl], y[sl]])})
    res = run_bass_kernel_spmd(_get_nc(), in_maps, list(range(N_CORES)))
    return np.concatenate([m["out"] for m in res.results])



# revision 4
# speedup vs baseline: 1.0205x; 1.0205x over previous
"""EMD loss kernel for Trainium2 (8 NeuronCores, pure data parallel).

Computes out[b] = sum_t (cumsum(x-y, axis=1)[b, t])^2 for x, y [131072, 256] f32.

Transposed matmul design (v2). The old row-major design was a three-way tie
(DVE tensor_tensor_scan 667ns/block = 85us, ACT square+accum 739ns/block =
95us, f32 DMA 83us). This version:
  - quantizes per-tensor to uint8 host-side (qx = round(256x),
    qny = 256 - round(256y)) and uploads bins-on-partitions transposed
    layout: 8.4 MB/core HBM traffic instead of 33.5 MB.
  - SWDGE cast+accum DMAs build z = qx + qny in fp16 SBUF (z-256 = qx-qy
    per bin, exact small ints in fp16).
  - PE computes the cumulative sums as triangular matmuls (3 per 512-row
    chunk: C1 = U^T z1; C2 = U^T z2 + ONES^T z1), killing the DVE scan.
  - ACT squares PSUM->SBUF fp16 with the -256*(t+1) ramp folded into the
    per-partition bias and the 1/256 scale folded into the activation scale:
    sq = Square(C_raw/256 - (t+1)) is exact in f32 up to the final rounding.
  - PE reduces over bins with a ones-vector matmul into a [1, 512] PSUM
    accumulator per chunk; DVE copies it to a partition-0 staging row.
"""

import numpy as np

from concourse import bacc, bass, mybir
from concourse.bass_utils import run_bass_kernel_spmd
from concourse.masks import make_upper_triangular
from concourse.tile import TileContext

N_CORES = 8
B = 131072
BINS = 256
ROWS = B // N_CORES  # 16384 rows per core
P = 128
CH = 2048  # strip width (rows per DMA pair)
N_STRIPS = ROWS // CH  # 8
NCH = 512  # matmul moving free dim (chunk)
CPS = CH // NCH  # chunks per strip = 4
N_CHUNK = ROWS // NCH  # 32

F32 = mybir.dt.float32
F16 = mybir.dt.float16
U8 = mybir.dt.uint8


def build_nc() -> bass.Bass:
    nc = bacc.Bacc()

    xy = nc.declare_dram_parameter("xy", [2, 2, P, ROWS], U8, isOutput=False)
    bias = nc.declare_dram_parameter("bias", [P, 2], F32, isOutput=False)
    out = nc.declare_dram_parameter("out", [ROWS], F32, isOutput=True)

    # [p, t, h, r]: partition = bin-within-half, free = (tensor, half, rows)
    xv = xy[:].rearrange("t h p r -> p t h r")

    with (
        TileContext(nc) as tc,
        tc.tile_pool(name="io", bufs=3) as io_pool,
        tc.tile_pool(name="sq", bufs=4) as sq_pool,
        tc.tile_pool(name="cpsum", bufs=2, space="PSUM") as c_pool,
        tc.tile_pool(name="spsum", bufs=2, space="PSUM") as s_pool,
        tc.tile_pool(name="const", bufs=1) as const_pool,
    ):
        # Constants: upper-triangular (incl. diag) U and all-ones, fp16.
        U = const_pool.tile([P, P], F16, tag="U")
        ONES = const_pool.tile([P, P], F16, tag="ONES")
        make_upper_triangular(nc, U[:], val=1.0, diag=True)
        nc.gpsimd.memset(ONES[:], 1.0)
        bias_sb = const_pool.tile([P, 2], F32, tag="bias")
        nc.sync.dma_start(out=bias_sb[:], in_=bias[:])
        stage = const_pool.tile([1, ROWS // NCH, NCH], F32, tag="stage")

        # Warm the ACT Square table so the ~1.3us table load overlaps the
        # first input DMAs.
        warm = const_pool.tile([P, 1], F32, tag="warm")
        warm2 = const_pool.tile([P, 1], F32, tag="warm2")
        nc.vector.memset(warm[:], 0)
        nc.scalar.activation(
            out=warm2[:], in_=warm[:], func=mybir.ActivationFunctionType.Square
        )

        for si in range(N_STRIPS):
            r0 = si * CH
            z = io_pool.tile([P, 2 * CH], F16, tag="z", name=f"z{si}")
            z3 = z[:].rearrange("p (h c) -> p h c", h=2)
            # qx (cast u8->fp16), then += qny via the CCE accumulate path.
            nc.gpsimd.dma_start(out=z3, in_=xv[:, 0, :, r0 : r0 + CH])
            nc.gpsimd.dma_start(
                out=z3,
                in_=xv[:, 1, :, r0 : r0 + CH],
                accum_op=mybir.AluOpType.add,
            )
            for ci in range(CPS):
                chunk = si * CPS + ci
                c0 = ci * NCH
                z1 = z[:, c0 : c0 + NCH]
                z2 = z[:, CH + c0 : CH + c0 + NCH]
                C1 = c_pool.tile([P, NCH], F32, tag="C1")
                C2 = c_pool.tile([P, NCH], F32, tag="C2")
                nc.tensor.matmul(C1[:], U[:], z1, start=True, stop=True)
                nc.tensor.matmul(C2[:], U[:], z2, start=True, stop=False)
                nc.tensor.matmul(C2[:], ONES[:], z1, start=False, stop=True)
                sq1 = sq_pool.tile([P, NCH], F16, tag="sq1")
                sq2 = sq_pool.tile([P, NCH], F16, tag="sq2")
                nc.scalar.activation(
                    out=sq1[:],
                    in_=C1[:],
                    func=mybir.ActivationFunctionType.Square,
                    scale=1.0 / 256.0,
                    bias=bias_sb[:, 0:1],
                )
                nc.scalar.activation(
                    out=sq2[:],
                    in_=C2[:],
                    func=mybir.ActivationFunctionType.Square,
                    scale=1.0 / 256.0,
                    bias=bias_sb[:, 1:2],
                )
                S = s_pool.tile([1, NCH], F32, tag="S")
                nc.tensor.matmul(S[:], ONES[:, 0:1], sq1[:], start=True, stop=False)
                nc.tensor.matmul(S[:], ONES[:, 0:1], sq2[:], start=False, stop=True)
                nc.vector.tensor_copy(stage[:, chunk, :], S[:])
        nc.sync.dma_start(
            out=out[:].rearrange("(o n c) -> o n c", o=1, c=NCH), in_=stage[:]
        )
    nc.finalize()
    return nc


_NC = None


def _get_nc() -> bass.Bass:
    global _NC
    if _NC is None:
        _NC = build_nc()
    return _NC


_BIAS = None


def _bias_arr() -> np.ndarray:
    global _BIAS
    if _BIAS is None:
        b = np.zeros((P, 2), np.float32)
        b[:, 0] = -(np.arange(P) + 1.0)
        b[:, 1] = -(np.arange(P) + 129.0)
        _BIAS = b
    return _BIAS


def make_in_maps(x: np.ndarray, y: np.ndarray) -> list[dict]:
    qx = np.clip(np.rint(x * 256.0), 0, 255).astype(np.uint8)
    qny = np.clip(256.0 - np.rint(y * 256.0), 0, 255).astype(np.uint8)
    bias = _bias_arr()
    in_maps = []
    for i in range(N_CORES):
        sl = slice(i * ROWS, (i + 1) * ROWS)
        xt = np.ascontiguousarray(qx[sl].T).reshape(2, P, ROWS)
        nyt = np.ascontiguousarray(qny[sl].T).reshape(2, P, ROWS)
        in_maps.append({"xy": np.stack([xt, nyt]), "bias": bias})
    return in_maps


def kernel(x: np.ndarray, y: np.ndarray) -> np.ndarray:
    assert x.shape == (B, BINS) and y.shape == (B, BINS), (x.shape, y.shape)
    x = np.ascontiguousarray(x, dtype=np.float32)
    y = np.ascontiguousarray(y, dtype=np.float32)
    res = run_bass_kernel_spmd(_get_nc(), make_in_maps(x, y), list(range(N_CORES)))
    return np.concatenate([m["out"] for m in res.results])


# revision 5
# speedup vs baseline: 1.3222x; 1.2957x over previous
"""EMD loss kernel for Trainium2 (8 NeuronCores, pure data parallel).

Computes out[b] = sum_t (cumsum(x-y, axis=1)[b, t])^2 for x, y [131072, 256] f32.

Transposed matmul design (v3). The row-major scan design was a three-way tie
(DVE tensor_tensor_scan 667ns/block = 85us, ACT square+accum 739ns/block =
95us, f32 DMA 83us, exec 101us). This version:
  - uploads x and -y as fp16 in a bins-on-partitions transposed layout
    (16.8 MB/core, one packed HWDGE DMA per 2048-row strip).
  - DVE pre-adds z = x + (-y) in fp16 2x mode (z1 = bins 0..127 on the 128
    partitions, z2 = bins 128..255).
  - PE computes the cumulative-sum differences as triangular matmuls
    (C1 = U^T z1; C2 = U^T z2 + ONES^T z1; 3 matmuls per 512-row chunk),
    replacing the unscalable DVE scan.
  - ACT squares two chunks at a time PSUM->SBUF fp16 ([128, 1024] tiles to
    amortize the 172-cycle PSUM access bubble).
  - PE ones-vector matmuls reduce over bins into [1, 512] PSUM rows; two
    chunks share one S bank at partition offsets {0, 64}, halving the DVE
    PSUM->SBUF copies. A single final DMA scatters the staging tile to DRAM.
"""

import numpy as np

from concourse import bacc, bass, mybir
from concourse.bass_utils import run_bass_kernel_spmd
from concourse.masks import make_upper_triangular
from concourse.tile import TileContext

N_CORES = 8
B = 131072
BINS = 256
ROWS = B // N_CORES  # 16384 rows per core
P = 128
CH = 2048  # strip width (rows per input DMA)
N_STRIPS = ROWS // CH  # 8
NCH = 512  # matmul moving free dim (chunk)
SUP = 1024  # super-chunk: ACT square granularity (2 chunks)
N_SUP = ROWS // SUP  # 16

F32 = mybir.dt.float32
F16 = mybir.dt.float16


def build_nc() -> bass.Bass:
    nc = bacc.Bacc()

    xy = nc.declare_dram_parameter("xy", [2, 2, P, ROWS], F16, isOutput=False)
    out = nc.declare_dram_parameter("out", [ROWS], F32, isOutput=True)

    # [p, (t h), r]: partition = bin-within-half, free = (tensor*half, rows)
    xv = xy[:].rearrange("t h p r -> p (t h) r")

    with (
        TileContext(nc) as tc,
        tc.tile_pool(name="io", bufs=2) as io_pool,
        tc.tile_pool(name="zp", bufs=2) as z_pool,
        tc.tile_pool(name="sq", bufs=2) as sq_pool,
        tc.tile_pool(name="c1p", bufs=2, space="PSUM") as c1_pool,
        tc.tile_pool(name="c2p", bufs=1, space="PSUM") as c2_pool,
        tc.tile_pool(name="sp", bufs=2, space="PSUM") as s_pool,
        tc.tile_pool(name="const", bufs=1) as const_pool,
    ):
        U = const_pool.tile([P, P], F16, tag="U")
        ONES = const_pool.tile([P, P], F16, tag="ONES")
        make_upper_triangular(nc, U[:], val=1.0, diag=True)
        nc.gpsimd.memset(ONES[:], 1.0)
        stage = const_pool.tile([P, N_SUP, NCH], F32, tag="stage")

        # Warm the ACT Square table so the ~1.3us table load overlaps the
        # first input DMAs.
        warm = const_pool.tile([P, 1], F32, tag="warm")
        warm2 = const_pool.tile([P, 1], F32, tag="warm2")
        nc.vector.memset(warm[:], 0)
        nc.scalar.activation(
            out=warm2[:], in_=warm[:], func=mybir.ActivationFunctionType.Square
        )

        for si in range(N_STRIPS):
            r0 = si * CH
            raw = io_pool.tile([P, 4 * CH], F16, tag="raw", name=f"raw{si}")
            raw3 = raw[:].rearrange("p (q c) -> p q c", q=4)
            nc.sync.dma_start(out=raw3, in_=xv[:, :, r0 : r0 + CH])
            z = z_pool.tile([P, 2 * CH], F16, tag="z", name=f"z{si}")
            # z1 = x1 - y1 (bins 0..127), z2 = x2 - y2 (bins 128..255)
            nc.vector.tensor_tensor(
                out=z[:, :CH],
                in0=raw[:, :CH],
                in1=raw[:, 2 * CH : 3 * CH],
                op=mybir.AluOpType.add,
            )
            nc.vector.tensor_tensor(
                out=z[:, CH:],
                in0=raw[:, CH : 2 * CH],
                in1=raw[:, 3 * CH :],
                op=mybir.AluOpType.add,
            )
            for ui in range(CH // SUP):
                sup = si * (CH // SUP) + ui
                c0 = ui * SUP
                C1 = c1_pool.tile([P, SUP], F32, tag="C1")
                C2 = c2_pool.tile([P, SUP], F32, tag="C2")
                for k in range(2):  # two 512-row chunks per super-chunk
                    z1 = z[:, c0 + k * NCH : c0 + (k + 1) * NCH]
                    z2 = z[:, CH + c0 + k * NCH : CH + c0 + (k + 1) * NCH]
                    ck = slice(k * NCH, (k + 1) * NCH)
                    nc.tensor.matmul(C1[:, ck], U[:], z1, start=True, stop=True)
                    nc.tensor.matmul(C2[:, ck], U[:], z2, start=True, stop=False)
                    nc.tensor.matmul(C2[:, ck], ONES[:], z1, start=False, stop=True)
                sq1 = sq_pool.tile([P, SUP], F16, tag="sq1")
                sq2 = sq_pool.tile([P, SUP], F16, tag="sq2")
                nc.scalar.activation(
                    out=sq1[:], in_=C1[:], func=mybir.ActivationFunctionType.Square
                )
                nc.scalar.activation(
                    out=sq2[:], in_=C2[:], func=mybir.ActivationFunctionType.Square
                )
                # Reduce over bins: chunk 2u -> S partition 0, 2u+1 -> 64.
                S = s_pool.tile([P, NCH], F32, tag="S")
                for k in range(2):
                    ck = slice(k * NCH, (k + 1) * NCH)
                    off = 64 * k
                    nc.tensor.matmul(
                        S[off : off + 1, :], ONES[:, 0:1], sq1[:, ck],
                        start=True, stop=False,
                    )
                    nc.tensor.matmul(
                        S[off : off + 1, :], ONES[:, 0:1], sq2[:, ck],
                        start=False, stop=True,
                    )
                nc.vector.tensor_copy(stage[:, sup, :], S[:])
        # stage rows {0, 64} of staging slot u hold chunks 2u and 2u+1.
        ov = out[:].rearrange("(n two c) -> two n c", two=2, c=NCH)
        nc.sync.dma_start(out=ov[0:1], in_=stage[0:1, :, :])
        nc.sync.dma_start(out=ov[1:2], in_=stage[64:65, :, :])
    nc.finalize()
    return nc


_NC = None


def _get_nc() -> bass.Bass:
    global _NC
    if _NC is None:
        _NC = build_nc()
    return _NC


def make_in_maps(x: np.ndarray, y: np.ndarray) -> list[dict]:
    x16 = x.astype(np.float16)
    ny16 = (-y).astype(np.float16)
    in_maps = []
    for i in range(N_CORES):
        sl = slice(i * ROWS, (i + 1) * ROWS)
        xt = np.ascontiguousarray(x16[sl].T).reshape(2, P, ROWS)
        nyt = np.ascontiguousarray(ny16[sl].T).reshape(2, P, ROWS)
        in_maps.append({"xy": np.stack([xt, nyt])})
    return in_maps


def kernel(x: np.ndarray, y: np.ndarray) -> np.ndarray:
    assert x.shape == (B, BINS) and y.shape == (B, BINS), (x.shape, y.shape)
    x = np.ascontiguousarray(x, dtype=np.float32)
    y = np.ascontiguousarray(y, dtype=np.float32)
    res = run_bass_kernel_spmd(_get_nc(), make_in_maps(x, y), list(range(N_CORES)))
    return np.concatenate([m["out"] for m in res.results])


# revision 9
# speedup vs baseline: 1.3339x; 1.0088x over previous
"""EMD loss kernel for Trainium2 (8 NeuronCores, pure data parallel).

Computes out[b] = sum_t (cumsum(x-y, axis=1)[b, t])^2 for x, y [131072, 256] f32.

Transposed matmul design (v3). The row-major scan design was a three-way tie
(DVE tensor_tensor_scan 667ns/block = 85us, ACT square+accum 739ns/block =
95us, f32 DMA 83us, exec 101us). This version:
  - uploads x and -y as fp16 in a bins-on-partitions transposed layout
    (16.8 MB/core, one packed HWDGE DMA per 2048-row strip).
  - DVE pre-adds z = x + (-y) in fp16 2x mode (z1 = bins 0..127 on the 128
    partitions, z2 = bins 128..255).
  - PE computes the cumulative-sum differences as triangular matmuls
    (C1 = U^T z1; C2 = U^T z2 + ONES^T z1; 3 matmuls per 512-row chunk),
    replacing the unscalable DVE scan.
  - ACT squares two chunks at a time PSUM->SBUF fp16 ([128, 1024] tiles to
    amortize the 172-cycle PSUM access bubble).
  - PE ones-vector matmuls reduce over bins into [1, 512] PSUM rows; two
    chunks share one S bank at partition offsets {0, 64}, halving the DVE
    PSUM->SBUF copies. A single final DMA scatters the staging tile to DRAM.
"""

import numpy as np

from concourse import bacc, bass, mybir
from concourse.bass_utils import run_bass_kernel_spmd
from concourse.masks import make_upper_triangular
from concourse.tile import TileContext

N_CORES = 8
B = 131072
BINS = 256
ROWS = B // N_CORES  # 16384 rows per core
P = 128
CH = 2048  # strip width (rows per input DMA)
N_STRIPS = ROWS // CH  # 8
NCH = 512  # matmul moving free dim (chunk)
SUP = 1024  # super-chunk: ACT square granularity (2 chunks)
N_SUP = ROWS // SUP  # 16

F32 = mybir.dt.float32
F16 = mybir.dt.float16


def build_nc() -> bass.Bass:
    nc = bacc.Bacc()

    # Strip-major host layout: per (partition, strip) all four quadrants
    # (x/ny x binhalf) are contiguous, so each strip DMA is one 16 KB run
    # per partition instead of four 4 KB runs.
    xy = nc.declare_dram_parameter(
        "xy", [P, N_STRIPS, 4, CH], F16, isOutput=False
    )
    out = nc.declare_dram_parameter("out", [ROWS], F32, isOutput=True)
    xv = xy[:]

    with (
        TileContext(nc) as tc,
        tc.tile_pool(name="io", bufs=2) as io_pool,
        tc.tile_pool(name="zp", bufs=2) as z_pool,
        tc.tile_pool(name="sq", bufs=2) as sq_pool,
        tc.tile_pool(name="c1p", bufs=2, space="PSUM") as c1_pool,
        tc.tile_pool(name="c2p", bufs=1, space="PSUM") as c2_pool,
        tc.tile_pool(name="sp", bufs=2, space="PSUM") as s_pool,
        tc.tile_pool(name="const", bufs=1) as const_pool,
    ):
        U = const_pool.tile([P, P], F16, tag="U")
        ONES = const_pool.tile([P, P], F16, tag="ONES")
        make_upper_triangular(nc, U[:], val=1.0, diag=True)
        nc.gpsimd.memset(ONES[:], 1.0)
        stage = const_pool.tile([P, N_SUP, NCH], F32, tag="stage")

        # Warm the ACT Square table so the ~1.3us table load overlaps the
        # first input DMAs.
        warm = const_pool.tile([P, 1], F32, tag="warm")
        warm2 = const_pool.tile([P, 1], F32, tag="warm2")
        nc.vector.memset(warm[:], 0)
        nc.scalar.activation(
            out=warm2[:], in_=warm[:], func=mybir.ActivationFunctionType.Square
        )

        for si in range(N_STRIPS):
            r0 = si * CH
            raw = io_pool.tile([P, 4 * CH], F16, tag="raw", name=f"raw{si}")
            raw3 = raw[:].rearrange("p (q c) -> p q c", q=4)
            nc.sync.dma_start(out=raw3, in_=xv[:, si, :, :])
            z = z_pool.tile([P, 2 * CH], F16, tag="z", name=f"z{si}")
            # z1 = x1 - y1 (bins 0..127), z2 = x2 - y2 (bins 128..255)
            nc.vector.tensor_tensor(
                out=z[:, :CH],
                in0=raw[:, :CH],
                in1=raw[:, 2 * CH : 3 * CH],
                op=mybir.AluOpType.add,
            )
            nc.vector.tensor_tensor(
                out=z[:, CH:],
                in0=raw[:, CH : 2 * CH],
                in1=raw[:, 3 * CH :],
                op=mybir.AluOpType.add,
            )
            for ui in range(CH // SUP):
                sup = si * (CH // SUP) + ui
                c0 = ui * SUP
                C1 = c1_pool.tile([P, SUP], F32, tag="C1")
                C2 = c2_pool.tile([P, SUP], F32, tag="C2")
                # All U-stationary matmuls first, then the ONES ones, so the
                # PE does 2 stationary swaps per super-chunk instead of 6.
                for k in range(2):
                    z2 = z[:, CH + c0 + k * NCH : CH + c0 + (k + 1) * NCH]
                    ck = slice(k * NCH, (k + 1) * NCH)
                    nc.tensor.matmul(
                        C1[:, ck], U[:], z[:, c0 + k * NCH : c0 + (k + 1) * NCH],
                        start=True, stop=True,
                    )
                    nc.tensor.matmul(C2[:, ck], U[:], z2, start=True, stop=False)
                for k in range(2):
                    z1 = z[:, c0 + k * NCH : c0 + (k + 1) * NCH]
                    ck = slice(k * NCH, (k + 1) * NCH)
                    nc.tensor.matmul(C2[:, ck], ONES[:], z1, start=False, stop=True)
                sq1 = sq_pool.tile([P, SUP], F16, tag="sq1")
                sq2 = sq_pool.tile([P, SUP], F16, tag="sq2")
                nc.scalar.activation(
                    out=sq1[:], in_=C1[:], func=mybir.ActivationFunctionType.Square
                )
                nc.scalar.activation(
                    out=sq2[:], in_=C2[:], func=mybir.ActivationFunctionType.Square
                )
                # Reduce over bins: chunk 2u -> S partition 0, 2u+1 -> 64.
                S = s_pool.tile([P, NCH], F32, tag="S")
                for k in range(2):
                    ck = slice(k * NCH, (k + 1) * NCH)
                    off = 64 * k
                    nc.tensor.matmul(
                        S[off : off + 1, :], ONES[:, 0:1], sq1[:, ck],
                        start=True, stop=False,
                    )
                    nc.tensor.matmul(
                        S[off : off + 1, :], ONES[:, 0:1], sq2[:, ck],
                        start=False, stop=True,
                    )
                nc.vector.tensor_copy(stage[:, sup, :], S[:])
        # stage rows {0, 64} of staging slot u hold chunks 2u and 2u+1.
        ov = out[:].rearrange("(n two c) -> two n c", two=2, c=NCH)
        nc.sync.dma_start(out=ov[0:1], in_=stage[0:1, :, :])
        nc.sync.dma_start(out=ov[1:2], in_=stage[64:65, :, :])
    nc.finalize()
    return nc


_NC = None


def _get_nc() -> bass.Bass:
    global _NC
    if _NC is None:
        _NC = build_nc()
    return _NC


def make_in_maps(x: np.ndarray, y: np.ndarray) -> list[dict]:
    x16 = x.astype(np.float16)
    ny16 = (-y).astype(np.float16)
    in_maps = []
    for i in range(N_CORES):
        sl = slice(i * ROWS, (i + 1) * ROWS)
        # [2(t), 2(h), P, ROWS] -> strip-major [P, N_STRIPS, 4(t*h), CH]
        xt = np.ascontiguousarray(x16[sl].T).reshape(2, P, ROWS)
        nyt = np.ascontiguousarray(ny16[sl].T).reshape(2, P, ROWS)
        q = np.stack([xt, nyt]).reshape(4, P, N_STRIPS, CH)
        in_maps.append({"xy": np.ascontiguousarray(q.transpose(1, 2, 0, 3))})
    return in_maps


def kernel(x: np.ndarray, y: np.ndarray) -> np.ndarray:
    assert x.shape == (B, BINS) and y.shape == (B, BINS), (x.shape, y.shape)
    x = np.ascontiguousarray(x, dtype=np.float32)
    y = np.ascontiguousarray(y, dtype=np.float32)
    res = run_bass_kernel_spmd(_get_nc(), make_in_maps(x, y), list(range(N_CORES)))
    return np.concatenate([m["out"] for m in res.results])
